# revision 22
# baseline (speedup 1.0000x reference)
"""Chunked (= full, non-causal) multi-head self-attention on 8 TRN2 NeuronCores.

Problem: B=2, S=2048, D=1024, H=16 heads (head_dim 64), torch-Linear-style
projections (y = x @ W.T + b), softmax attention, output projection.

Sharding: head-parallel. Core c owns heads {2c, 2c+1} = feature slice
[128c, 128c+128). Each core computes q/k/v for its slice from the full x
(replicated), runs attention for its 4 (batch, head) pairs, and produces a
partial output projection with its 128-row slice of Wo. Host sums the 8
partials (bf16) and adds bo.

Layout: scores are computed transposed, ST[k, q] (keys on partitions), so the
softmax exp output PT feeds the P@V matmul directly (contraction over k on
partitions) — x and the weights are pre-transposed AND pre-tiled on the host
so every DMA line is per-partition contiguous (the naive gather layouts ran
at 256B-1KB lines and stalled the kernel start ~15us). The two heads' K=64
score matmuls land on PE row-groups 0-1/2-3. The softmax denominator rides as
row 64 of the PV output via a ones-column appended to V (M=65). V is computed
feature-major (N=512 matmuls) then PE-transposed per 128-token chunk into the
PV layout: the direct token-major form (N=128 matmuls) was LDWEIGHTS-bound at
~219ns/matmul.

Engine balance (the v1 kernel was ACT-bound: 16.8M softmax exps at 1
elem/lane/cycle = ~147us on ScalarE alone):
  - exp is SPLIT between ACT (accurate spline exp) and DVE (Schraudolph
    bit-trick exp in bf16 space: bf16_bits(e^s) ~= round(s * 128/ln2 +
    (127*128 - 7.5)), computed as ONE fp32 tensor_scalar with int16 output,
    then the int16 tile is bitcast to bf16). Per-tile round-robin with a
    DVE-fraction knob; rel-err contribution ~1.3e-2 at 50% DVE.
  - softmax reciprocal via DVE reciprocal_approx_fast (kills the v1 ACT
    Ln/Exp chain and the activation-table monkeypatch).
  - q/k bias adds, v evacuation (bias folded into a rank-1 ones matmul) and
    o-raw evacuations moved to the otherwise-idle ScalarE (Identity/Copy are
    in the exp table set -> no table swaps); y evacuations split ACT/DVE.
  - ~20 tiny warm-up matmuls at t=0 keep the PE HAM monitor busy through the
    initial DMA wait so real matmuls run at 2.4GHz, not 1.2.

Precision: bf16 in, fp32 accumulate, bf16 partial-y out (~1.4e-2 rel err).
"""

import sys

if "/opt/trn_rl_repo" not in sys.path:
    sys.path.insert(0, "/opt/trn_rl_repo")

import numpy as np

import concourse.bacc as bacc
import concourse.mybir as mybir
import concourse.tile as tile
from concourse import bass_utils

B, S, D, H = 2, 2048, 1024, 16
HD = D // H          # 64
NCORES = 8
ES = D // NCORES     # 128 features (= 2 heads) per core
BS = B * S           # 4096 rows total

P = 128              # partitions
NF = 512             # matmul free-dim tile
N_SB = BS // NF      # 8 s-blocks of 512
N_DC = D // P        # 8 contraction chunks of 128
N_KB = S // P        # 16 key blocks of 128 per batch
N_KP = N_KB // 2     # 8 key-block PAIRS per batch
N_QC = S // NF       # 4 query chunks of 512 per batch
N_CH = BS // P       # 32 global 128-row chunks

F32 = mybir.dt.float32
BF16 = mybir.dt.bfloat16
I16 = mybir.dt.int16

DT_QK = BF16         # x/Wq/Wk inputs for q,k projections + score matmuls
DT_V = BF16          # x/Wv inputs for v projection
DT_ATT = BF16        # attention weights (exp output) and V in the P@V matmul
DT_OUT = BF16        # output projection inputs (OT, Wo)

# ---- tuning knobs -----------------------------------------------------------
FRAC_DVE_EXP = 0.56  # fraction of exp tiles done on DVE via Schraudolph
FRAC_ACT_Y = 0.42    # fraction of y evacuations done on ACT
N_WARMUP_MM = 70     # tiny matmuls at t=0 to keep the PE HAM monitor warm

# Schraudolph-in-bf16: bits = round(s * (1/sqrt(HD)) * 128/ln2 + (127*128 - C))
_INV_SQRT_HD = 1.0 / float(np.sqrt(HD))
SCHRAUD_A = float(_INV_SQRT_HD * 128.0 / np.log(2.0))
SCHRAUD_B = float(127.0 * 128.0 - 7.5)  # C=7.5 rms-optimal; HW rounds to nearest

DEBUG = False

_cache = {}
last_results = None          # test.py reads exec_time_ns off this


def _np_dt(dt):
    import ml_dtypes

    return np.dtype(ml_dtypes.bfloat16) if dt == mybir.dt.bfloat16 else np.dtype(np.float32)


def _build():
    nc = bacc.Bacc("TRN2", target_bir_lowering=False, debug=False)

    # x / W layouts pre-arranged on the host so every DMA line is
    # per-partition contiguous (8KB strips, 2KB weights): the naive
    # [D, BS] gather ran at ~256B-1KB per line and stalled the PE ~15us
    # at kernel start.
    xT_d = nc.dram_tensor("xT", [P, N_SB, N_DC, NF], DT_QK, kind="ExternalInput")
    wqT_d = nc.dram_tensor("wqT", [P, N_DC, ES], DT_QK, kind="ExternalInput")
    wkT_d = nc.dram_tensor("wkT", [P, N_DC, ES], DT_QK, kind="ExternalInput")
    wvT_d = nc.dram_tensor("wvT", [P, N_DC, ES], DT_V, kind="ExternalInput")
    bq_d = nc.dram_tensor("bq", [ES, 1], F32, kind="ExternalInput")
    bk_d = nc.dram_tensor("bk", [ES, 1], F32, kind="ExternalInput")
    bv_d = nc.dram_tensor("bv", [ES, 1], F32, kind="ExternalInput")
    eye_d = nc.dram_tensor("eye", [P, P], DT_V, kind="ExternalInput")
    woT_d = nc.dram_tensor("woT", [ES, D], DT_OUT, kind="ExternalInput")
    y_d = nc.dram_tensor("y", [BS, D], BF16, kind="ExternalOutput")
    if DEBUG:
        qT_dbg = nc.dram_tensor("qT_dbg", [P, BS], DT_QK, kind="ExternalOutput")
        kT_dbg = nc.dram_tensor("kT_dbg", [P, BS], DT_QK, kind="ExternalOutput")
        v_dbg = nc.dram_tensor("v_dbg", [P, N_CH * 2 * (HD + 1)], DT_ATT, kind="ExternalOutput")
        oT_dbg = nc.dram_tensor("oT_dbg", [P, BS], DT_OUT, kind="ExternalOutput")
        oraw_dbg = nc.dram_tensor("oraw_dbg", [HD + 1, 16 * 2 * NF], F32, kind="ExternalOutput")
        rcp_dbg = nc.dram_tensor("rcp_dbg", [1, 16 * 2 * NF], F32, kind="ExternalOutput")
        pt_dbg = nc.dram_tensor("pt_dbg", [P, 8 * 2 * NF], DT_ATT, kind="ExternalOutput")

    with tile.TileContext(nc) as tc:
        with tc.tile_pool(name="const", bufs=1) as cpool, \
             tc.tile_pool(name="xt", bufs=3) as xt_pool, \
             tc.tile_pool(name="qkv", bufs=1) as qkv_pool, \
             tc.tile_pool(name="pt", bufs=10) as pt_pool, \
             tc.tile_pool(name="ysb", bufs=6) as y_pool, \
             tc.tile_pool(name="ps", bufs=1, space="PSUM") as ps:

            # ---- PE warm-up: tiny matmuls while the first DMAs land -------
            dummy_w = cpool.tile([1, P], DT_QK)
            nc.vector.memset(dummy_w[:], 0.0)
            for _ in range(N_WARMUP_MM):
                warm_ps = ps.tile([P, P], F32, tag="misc", bufs=2)
                nc.tensor.matmul(warm_ps[:], dummy_w[:], dummy_w[:],
                                 start=True, stop=True)

            # ---- constants / weights ------------------------------------
            # (first-strip DMA is issued before these from the gpsimd queue
            # inside the batch-0 loop; weights ride the sync/scalar queues)
            wk_sb = cpool.tile([P, N_DC, ES], DT_QK)
            wq_sb = cpool.tile([P, N_DC, ES], DT_QK)
            wv_sb = cpool.tile([P, N_DC, ES], DT_V)
            nc.sync.dma_start(wk_sb[:], wkT_d[:])
            bk_sb = cpool.tile([ES, 1], F32)
            bq_sb = cpool.tile([ES, 1], F32)
            nc.scalar.dma_start(bk_sb[:], bk_d[:])
            nc.sync.dma_start(wq_sb[:], wqT_d[:])
            nc.scalar.dma_start(bq_sb[:], bq_d[:])
            nc.sync.dma_start(wv_sb[:], wvT_d[:])
            bv_sb = cpool.tile([ES, 1], F32)
            nc.scalar.dma_start(bv_sb[:], bv_d[:])
            wo_sb = cpool.tile([ES, D], DT_OUT)
            nc.gpsimd.dma_start(wo_sb[:], woT_d[:])
            eye_sb = cpool.tile([P, P], DT_V)
            nc.scalar.dma_start(eye_sb[:], eye_d[:])

            # ---- persistent activations ---------------------------------
            qT_sb = qkv_pool.tile([P, BS], DT_QK)     # [feat 128, s 4096]
            kT_sb = qkv_pool.tile([P, BS], DT_QK)
            vT_sb = qkv_pool.tile([P, BS], DT_V)      # [feat 128, s 4096]
            # V for both heads + ones col: [tok, chunk, head, HD+1]
            v_sb = qkv_pool.tile([P, N_CH, 2, HD + 1], DT_ATT)
            oT_sb = qkv_pool.tile([P, BS], DT_OUT)    # normalized attn out, [feat, s]
            nc.vector.memset(v_sb[:, :, :, HD : HD + 1], 1.0)


            # ---- emission helpers ---------------------------------------
            strips = {}

            def emit_strip_dma(sb):
                strip = xt_pool.tile([P, N_DC, NF], DT_QK, tag="strip", name=f"strip{sb}")
                eng = nc.gpsimd if sb <= 3 else nc.sync
                eng.dma_start(strip[:], xT_d[:, sb])
                strips[sb] = strip

            def emit_qk_piece(sb, which):
                s0 = sb * NF
                strip = strips[sb]
                w_sb, bias, dst = ((wq_sb, bq_sb, qT_sb) if which == "q"
                                   else (wk_sb, bk_sb, kT_sb))
                p_ps = ps.tile([P, NF], F32, tag="misc", bufs=2, name=f"{which}{sb}_ps")
                for j in range(N_DC):
                    nc.tensor.matmul(p_ps[:], w_sb[:, j], strip[:, j],
                                     start=(j == 0), stop=(j == N_DC - 1))
                # bias-add + PSUM->SBUF evacuation on the scalar engine
                nc.scalar.add(dst[:, s0 : s0 + NF], p_ps[:], bias[:])

            def emit_v_piece(sb):
                # feat-major vT projection (N=512 matmuls, same shape as
                # q/k — the old token-major N=128 form was LDWEIGHTS-bound
                # at ~219ns per matmul, ~2x the streaming cost)
                s0 = sb * NF
                strip = strips[sb]
                v_ps = ps.tile([P, NF], F32, tag="misc", bufs=2, name=f"v{sb}_ps")
                for j in range(N_DC):
                    nc.tensor.matmul(v_ps[:], wv_sb[:, j], strip[:, j],
                                     start=(j == 0), stop=(j == N_DC - 1))
                nc.scalar.add(vT_sb[:, s0 : s0 + NF], v_ps[:], bv_sb[:])

            def emit_v_trans(sb, ss):
                # PE-transpose one 128-token chunk of vT into PV layout
                ch = sb * (NF // P) + ss
                vtp = ps.tile([P, P], DT_V, tag="misc", bufs=2, name=f"vt{ch}_ps")
                nc.tensor.transpose(vtp[:], vT_sb[:, ch * P : (ch + 1) * P],
                                    eye_sb[:])
                nc.scalar.copy(
                    v_sb[:, ch, :, 0:HD],
                    vtp[:].rearrange("p (h f) -> p h f", h=2))


            inv_sqrt_hd = _INV_SQRT_HD
            y_queue = []
            exp_acc = [0.0]

            def emit_exp(st2, pt2):
                # one engine per [128, 2, 512] tile (2 PSUM banks); round-robin
                # weighted by FRAC_DVE_EXP
                exp_acc[0] += FRAC_DVE_EXP
                if exp_acc[0] >= 1.0:
                    exp_acc[0] -= 1.0
                    nc.vector.tensor_scalar(
                        out=pt2[:].bitcast(I16), in0=st2[:],
                        scalar1=SCHRAUD_A, scalar2=SCHRAUD_B,
                        op0=mybir.AluOpType.mult, op1=mybir.AluOpType.add)
                else:
                    nc.scalar.activation(pt2[:], st2[:],
                                         mybir.ActivationFunctionType.Exp,
                                         scale=inv_sqrt_hd)

            recip_idx = [0]

            def emit_recip_chain(oraw, q0, last=False):
                # 1/rowsum on DVE (approx, ~51 ULP), then ONE partition
                # broadcast for both heads on the idle GPSIMD
                # custom-DVE ops and partition_broadcast only honor
                # partition base 0, so first move the rowsum row (partition
                # 64) to a base-0 tile with a tiny SBUF->SBUF DMA (4KB,
                # off-engine), then 1/x on DVE and broadcast on GPSIMD.
                den0 = pt_pool.tile([1, 2, NF], F32, tag="den", bufs=4)
                if last:
                    # ACT is idle at the end; its cross-partition copy is
                    # lower-latency than the DMA round trip
                    nc.scalar.copy(den0[:], oraw[HD : HD + 1, :, :])
                else:
                    nc.sync.dma_start(den0[:], oraw[HD : HD + 1, :, :])
                rcp2 = pt_pool.tile([1, 2, NF], F32, tag="rcp", bufs=4)
                nc.vector.reciprocal_approx_fast(out=rcp2[:], in_=den0[:])
                bc2 = pt_pool.tile([HD, 2, NF], F32, tag="bc", bufs=3)
                nc.gpsimd.partition_broadcast(bc2[:], rcp2[:])
                if DEBUG:
                    di = recip_idx[0]
                    recip_idx[0] += 1
                    dsl = slice(di * 2 * NF, (di + 1) * 2 * NF)
                    nc.sync.dma_start(oraw_dbg[:, dsl],
                                      oraw[:].rearrange("p h f -> p (h f)"))
                    nc.sync.dma_start(rcp_dbg[:, dsl],
                                      bc2[0:1].rearrange("p h f -> p (h f)"))
                return (oraw, bc2, q0)

            def emit_apply(oraw, bc2, q0):
                for hidx, part in ((0, 0), (1, HD)):
                    nc.vector.tensor_mul(
                        oT_sb[part : part + HD, q0 : q0 + NF],
                        oraw[0:HD, hidx], bc2[:, hidx])
                for ss in range(NF // P):
                    for ec in range(D // NF):
                        y_queue.append((q0 + ss * P, ec))

            y_acc = [0.0]

            y_dma_eng = [0]

            def emit_yproj(s0, ec, tail=False):
                y_ps = ps.tile([P, NF], F32, tag="misc", bufs=2)
                nc.tensor.matmul(y_ps[:], oT_sb[:, s0 : s0 + P],
                                 wo_sb[:, ec * NF : (ec + 1) * NF],
                                 start=True, stop=True)
                y_sb = y_pool.tile([P, NF], BF16, tag="y")
                y_acc[0] += FRAC_ACT_Y
                if y_acc[0] >= 1.0:
                    y_acc[0] -= 1.0
                    nc.scalar.copy(y_sb[:], y_ps[:])
                else:
                    nc.vector.tensor_copy(y_sb[:], y_ps[:])
                if tail:
                    # spread the final burst of y writes over both DMA-capable
                    # idle queues so the ~610ns issue cost doesn't serialize
                    eng = (nc.sync, nc.gpsimd)[y_dma_eng[0] % 2]
                    y_dma_eng[0] += 1
                else:
                    eng = nc.sync
                eng.dma_start(y_d[s0 : s0 + P, ec * NF : (ec + 1) * NF], y_sb[:])

            # ---- projections for batch 0 (k/v first; q trails as filler) -
            for sb in range(N_SB // 2):
                emit_strip_dma(sb)
                emit_qk_piece(sb, "k")
                emit_v_piece(sb)
                if sb == 0:
                    emit_qk_piece(0, "q")
                if sb > 0:
                    for ss in range(NF // P):
                        emit_v_trans(sb - 1, ss)

            # filler work queues: remaining q pieces + batch-1 projections
            # drip-feed into batch-0 attention; deferred output projections
            # drip into batch-1. q_sb{i} must complete before (b0, qc=i).
            a_queue = [("vt", 3, 0), ("vt", 3, 1), ("vt", 3, 2), ("vt", 3, 3),
                       ("q", 1), ("q", 2), ("q", 3)]
            for sb in range(N_SB // 2, N_SB):
                a_queue.append(("dma", sb))
                a_queue.append(("q", sb))
                a_queue.append(("k", sb))
                a_queue.append(("v", sb))
                for ss in range(NF // P):
                    a_queue.append(("vt", sb, ss))

            def emit_a_piece():
                piece = a_queue.pop(0)
                if piece[0] == "dma":
                    emit_strip_dma(piece[1])
                    if a_queue:
                        emit_a_piece()  # dma is async; also emit a compute piece
                elif piece[0] in ("q", "k"):
                    emit_qk_piece(piece[1], piece[0])
                elif piece[0] == "v":
                    emit_v_piece(piece[1])
                else:
                    emit_v_trans(piece[1], piece[2])

            # ---- attention: one continuous software pipeline -------------
            # Global stream over 64 ST pair-slots (8 per (b,qc) iteration);
            # PV consumption lags ST/exp by one pair and crosses iteration
            # boundaries, so the PE pipeline never drains mid-kernel.
            n_iters = B * N_QC
            total_pairs = n_iters * N_KP
            o_tiles = {}
            ptq = {}
            pending = None
            norm_state = None

            for g in range(total_pairs + 1):
                if g < total_pairs:
                    it = g // N_KP
                    kp = g % N_KP
                    b, qc = it // N_QC, it % N_QC
                    if kp == 0 and b == 1 and qc == 0:
                        while a_queue:
                            emit_a_piece()
                    q0 = b * S + qc * NF
                    st2A = ps.tile([P, 2, NF], F32, tag="st2", bufs=2)
                    st2B = ps.tile([P, 2, NF], F32, tag="st2", bufs=2)
                    for half in range(2):
                        k0 = b * S + (kp * 2 + half) * P
                        nc.tensor.matmul(st2A[:, half], kT_sb[0:HD, k0 : k0 + P],
                                         qT_sb[0:HD, q0 : q0 + NF],
                                         start=True, stop=True)
                        nc.tensor.matmul(st2B[:, half], kT_sb[HD:P, k0 : k0 + P],
                                         qT_sb[HD:P, q0 : q0 + NF],
                                         start=True, stop=True)
                    pt2A = pt_pool.tile([P, 2, NF], DT_ATT, tag="pt", bufs=10)
                    pt2B = pt_pool.tile([P, 2, NF], DT_ATT, tag="pt", bufs=10)
                    emit_exp(st2A, pt2A)
                    emit_exp(st2B, pt2B)
                    ptq[g] = (pt2A, pt2B)
                    if DEBUG and g < 8:
                        nc.sync.dma_start(
                            pt_dbg[:, g * 2 * NF : (g + 1) * 2 * NF],
                            pt2A[:].rearrange("p h f -> p (h f)"))

                    # fillers ride the ST side of the stream
                    if b == 0:
                        if a_queue:
                            emit_a_piece()
                    else:
                        npop = 3 if it >= n_iters - 2 else 2
                        for _ in range(npop):
                            # hold 16 back: they fill the PE while the final
                            # normalize chain runs
                            if len(y_queue) > 16:
                                emit_yproj(*y_queue.pop(0))
                    if kp == 1 and pending is not None:
                        norm_state = emit_recip_chain(*pending)
                        pending = None
                    if kp == 4 and norm_state is not None:
                        emit_apply(*norm_state)
                        norm_state = None

                if g >= 1:
                    pg = g - 1
                    it = pg // N_KP
                    kp = pg % N_KP
                    b, qc = it // N_QC, it % N_QC
                    q0 = b * S + qc * NF
                    if kp == 0:
                        oA_new = ps.tile([HD + 1, NF], F32, tag="o", bufs=2)
                        oB_new = ps.tile([HD + 1, NF], F32, tag="o", bufs=2)
                        o_tiles[it] = (oA_new, oB_new)
                    oA_ps, oB_ps = o_tiles[it]
                    pt2A, pt2B = ptq.pop(pg)
                    for half in range(2):
                        kb = kp * 2 + half
                        gkb = b * N_KB + kb
                        nc.tensor.matmul(oA_ps[:], v_sb[:, gkb, 0], pt2A[:, half],
                                         start=(kb == 0), stop=(kb == N_KB - 1))
                        nc.tensor.matmul(oB_ps[:], v_sb[:, gkb, 1], pt2B[:, half],
                                         start=(kb == 0), stop=(kb == N_KB - 1))
                    if kp == N_KP - 1:
                        # iteration finished: evacuate raw o + rowsum on the
                        # scalar engine, free the PSUM banks, defer norm
                        oraw = pt_pool.tile([HD + 1, 2, NF], F32, tag="oraw", bufs=3)
                        nc.scalar.copy(oraw[:, 0], oA_ps[:])
                        nc.scalar.copy(oraw[:, 1], oB_ps[:])
                        del o_tiles[it]
                        pending = (oraw, q0)

            emit_apply(*emit_recip_chain(*pending, last=True))
            for s0, ec in y_queue:
                emit_yproj(s0, ec, tail=True)

            if DEBUG:
                nc.sync.dma_start(qT_dbg[:], qT_sb[:])
                nc.sync.dma_start(kT_dbg[:], kT_sb[:])
                nc.sync.dma_start(v_dbg[:], v_sb[:].rearrange("p a h e -> p (a h e)"))
                nc.sync.dma_start(oT_dbg[:], oT_sb[:])

    nc.compile()
    return nc


def kernel(x, Wq, bq, Wk, bk, Wv, bv, Wo, bo, _trace=False):
    global last_results
    x = np.asarray(x, dtype=np.float32)
    Wq, bq = np.asarray(Wq, np.float32), np.asarray(bq, np.float32)
    Wk, bk = np.asarray(Wk, np.float32), np.asarray(bk, np.float32)
    Wv, bv = np.asarray(Wv, np.float32), np.asarray(bv, np.float32)
    Wo, bo = np.asarray(Wo, np.float32), np.asarray(bo, np.float32)

    if "nc" not in _cache:
        _cache["nc"] = _build()
    nc = _cache["nc"]

    dt_qk, dt_v, dt_out = _np_dt(DT_QK), _np_dt(DT_V), _np_dt(DT_OUT)
    _EYE = np.eye(P, dtype=dt_v)
    # [P, N_SB, N_DC, NF]: strip DMA lines are per-partition contiguous 8KB
    xT_qk = np.ascontiguousarray(
        x.reshape(N_SB, NF, N_DC, P).transpose(3, 0, 2, 1)).astype(dt_qk, copy=False)
    in_maps = []
    for c in range(NCORES):
        sl = slice(c * ES, (c + 1) * ES)
        in_maps.append({
            "xT": xT_qk,
            "wqT": np.ascontiguousarray(
                Wq[sl].T.reshape(N_DC, P, ES).transpose(1, 0, 2)).astype(dt_qk, copy=False),
            "wkT": np.ascontiguousarray(
                Wk[sl].T.reshape(N_DC, P, ES).transpose(1, 0, 2)).astype(dt_qk, copy=False),
            "wvT": np.ascontiguousarray(
                Wv[sl].T.reshape(N_DC, P, ES).transpose(1, 0, 2)).astype(dt_v, copy=False),
            "bq": np.ascontiguousarray(bq[sl, None]),
            "bk": np.ascontiguousarray(bk[sl, None]),
            "bv": np.ascontiguousarray(bv[sl, None]),
            "eye": _EYE,
            "woT": np.ascontiguousarray(Wo[:, sl].T).astype(dt_out, copy=False),
        })

    res = bass_utils.run_bass_kernel_spmd(
        nc, in_maps, core_ids=list(range(NCORES)), trace=_trace)
    last_results = res

    y = res.results[0]["y"].astype(np.float64)
    for c in range(1, NCORES):
        y += res.results[c]["y"]
    y = (y + bo).astype(np.float32)
    return y.reshape(B, S, D)


# revision 23
# speedup vs baseline: 1.0254x; 1.0254x over previous
"""Chunked (= full, non-causal) multi-head self-attention on 8 TRN2 NeuronCores.

Problem: B=2, S=2048, D=1024, H=16 heads (head_dim 64), torch-Linear-style
projections (y = x @ W.T + b), softmax attention, output projection.

Sharding: head-parallel. Core c owns heads {2c, 2c+1} = feature slice
[128c, 128c+128). Each core computes q/k/v for its slice from the full x
(replicated), runs attention for its 4 (batch, head) pairs, and produces a
partial output projection with its 128-row slice of Wo. Host sums the 8
partials (bf16) and adds bo.

Layout: scores are computed transposed, ST[k, q] (keys on partitions), so the
softmax exp output PT feeds the P@V matmul directly (contraction over k on
partitions) — x and the weights are pre-transposed AND pre-tiled on the host
so every DMA line is per-partition contiguous (the naive gather layouts ran
at 256B-1KB lines and stalled the kernel start ~15us). The two heads' K=64
score matmuls land on PE row-groups 0-1/2-3. The softmax denominator rides as
row 64 of the PV output via a ones-column appended to V (M=65). V is computed
feature-major (N=512 matmuls) then PE-transposed per 128-token chunk into the
PV layout: the direct token-major form (N=128 matmuls) was LDWEIGHTS-bound at
~219ns/matmul.

Engine balance (the v1 kernel was ACT-bound: 16.8M softmax exps at 1
elem/lane/cycle = ~147us on ScalarE alone):
  - exp is SPLIT between ACT (accurate spline exp) and DVE (Schraudolph
    bit-trick exp in bf16 space: bf16_bits(e^s) ~= round(s * 128/ln2 +
    (127*128 - 7.5)), computed as ONE fp32 tensor_scalar with int16 output,
    then the int16 tile is bitcast to bf16). Per-tile round-robin with a
    DVE-fraction knob; rel-err contribution ~1.3e-2 at 50% DVE.
  - softmax reciprocal via DVE reciprocal_approx_fast (kills the v1 ACT
    Ln/Exp chain and the activation-table monkeypatch).
  - q/k bias adds, v evacuation (bias folded into a rank-1 ones matmul) and
    o-raw evacuations moved to the otherwise-idle ScalarE (Identity/Copy are
    in the exp table set -> no table swaps); y evacuations split ACT/DVE.
  - ~20 tiny warm-up matmuls at t=0 keep the PE HAM monitor busy through the
    initial DMA wait so real matmuls run at 2.4GHz, not 1.2.

Precision: bf16 in, fp32 accumulate, bf16 partial-y out (~1.4e-2 rel err).
"""

import sys

if "/opt/trn_rl_repo" not in sys.path:
    sys.path.insert(0, "/opt/trn_rl_repo")

import numpy as np

import concourse.bacc as bacc
import concourse.mybir as mybir
import concourse.tile as tile
from concourse import bass_utils

B, S, D, H = 2, 2048, 1024, 16
HD = D // H          # 64
NCORES = 8
ES = D // NCORES     # 128 features (= 2 heads) per core
BS = B * S           # 4096 rows total

P = 128              # partitions
NF = 512             # matmul free-dim tile
N_SB = BS // NF      # 8 s-blocks of 512
N_DC = D // P        # 8 contraction chunks of 128
N_KB = S // P        # 16 key blocks of 128 per batch
N_KP = N_KB // 2     # 8 key-block PAIRS per batch
N_QC = S // NF       # 4 query chunks of 512 per batch
N_CH = BS // P       # 32 global 128-row chunks

F32 = mybir.dt.float32
BF16 = mybir.dt.bfloat16
I16 = mybir.dt.int16

DT_QK = BF16         # x/Wq/Wk inputs for q,k projections + score matmuls
DT_V = BF16          # x/Wv inputs for v projection
DT_ATT = BF16        # attention weights (exp output) and V in the P@V matmul
DT_OUT = BF16        # output projection inputs (OT, Wo)

# ---- tuning knobs -----------------------------------------------------------
FRAC_DVE_EXP = 0.56  # fraction of exp tiles done on DVE via Schraudolph
FRAC_ACT_Y = 0.42    # fraction of y evacuations done on ACT
N_WARMUP_MM = 70     # tiny matmuls at t=0 to keep the PE HAM monitor warm

# Schraudolph-in-bf16: bits = round(s * (1/sqrt(HD)) * 128/ln2 + (127*128 - C))
_INV_SQRT_HD = 1.0 / float(np.sqrt(HD))
SCHRAUD_A = float(_INV_SQRT_HD * 128.0 / np.log(2.0))
SCHRAUD_B = float(127.0 * 128.0 - 7.5)  # C=7.5 rms-optimal; HW rounds to nearest

DEBUG = False

_cache = {}
last_results = None          # test.py reads exec_time_ns off this


def _np_dt(dt):
    import ml_dtypes

    return np.dtype(ml_dtypes.bfloat16) if dt == mybir.dt.bfloat16 else np.dtype(np.float32)


def _build():
    nc = bacc.Bacc("TRN2", target_bir_lowering=False, debug=False)

    # x / W layouts pre-arranged on the host so every DMA line is
    # per-partition contiguous (8KB strips, 2KB weights): the naive
    # [D, BS] gather ran at ~256B-1KB per line and stalled the PE ~15us
    # at kernel start.
    xT_d = nc.dram_tensor("xT", [P, N_SB, N_DC, NF], DT_QK, kind="ExternalInput")
    wqT_d = nc.dram_tensor("wqT", [P, N_DC, ES], DT_QK, kind="ExternalInput")
    wkT_d = nc.dram_tensor("wkT", [P, N_DC, ES], DT_QK, kind="ExternalInput")
    wvT_d = nc.dram_tensor("wvT", [P, N_DC, ES], DT_V, kind="ExternalInput")
    bq_d = nc.dram_tensor("bq", [ES, 1], F32, kind="ExternalInput")
    bk_d = nc.dram_tensor("bk", [ES, 1], F32, kind="ExternalInput")
    bv_d = nc.dram_tensor("bv", [ES, 1], F32, kind="ExternalInput")
    eye_d = nc.dram_tensor("eye", [P, P], DT_V, kind="ExternalInput")
    woT_d = nc.dram_tensor("woT", [ES, D], DT_OUT, kind="ExternalInput")
    y_d = nc.dram_tensor("y", [BS, D], BF16, kind="ExternalOutput")
    if DEBUG:
        qT_dbg = nc.dram_tensor("qT_dbg", [P, BS], DT_QK, kind="ExternalOutput")
        kT_dbg = nc.dram_tensor("kT_dbg", [P, BS], DT_QK, kind="ExternalOutput")
        v_dbg = nc.dram_tensor("v_dbg", [P, N_CH * 2 * (HD + 1)], DT_ATT, kind="ExternalOutput")
        oT_dbg = nc.dram_tensor("oT_dbg", [P, BS], DT_OUT, kind="ExternalOutput")
        oraw_dbg = nc.dram_tensor("oraw_dbg", [HD + 1, 16 * 2 * NF], F32, kind="ExternalOutput")
        rcp_dbg = nc.dram_tensor("rcp_dbg", [1, 16 * 2 * NF], F32, kind="ExternalOutput")
        pt_dbg = nc.dram_tensor("pt_dbg", [P, 8 * 2 * NF], DT_ATT, kind="ExternalOutput")

    with tile.TileContext(nc) as tc:
        with tc.tile_pool(name="const", bufs=1) as cpool, \
             tc.tile_pool(name="xt", bufs=3) as xt_pool, \
             tc.tile_pool(name="qkv", bufs=1) as qkv_pool, \
             tc.tile_pool(name="pt", bufs=10) as pt_pool, \
             tc.tile_pool(name="ysb", bufs=6) as y_pool, \
             tc.tile_pool(name="ps", bufs=1, space="PSUM") as ps:

            # ---- PE warm-up: tiny matmuls while the first DMAs land -------
            dummy_w = cpool.tile([1, P], DT_QK)
            nc.vector.memset(dummy_w[:], 0.0)
            for _ in range(N_WARMUP_MM):
                warm_ps = ps.tile([P, P], F32, tag="misc", bufs=2)
                nc.tensor.matmul(warm_ps[:], dummy_w[:], dummy_w[:],
                                 start=True, stop=True)

            # ---- constants / weights ------------------------------------
            # (first-strip DMA is issued before these from the gpsimd queue
            # inside the batch-0 loop; weights ride the sync/scalar queues)
            wk_sb = cpool.tile([P, N_DC, ES], DT_QK)
            wq_sb = cpool.tile([P, N_DC, ES], DT_QK)
            wv_sb = cpool.tile([P, N_DC, ES], DT_V)
            nc.sync.dma_start(wk_sb[:], wkT_d[:])
            bk_sb = cpool.tile([ES, 1], F32)
            bq_sb = cpool.tile([ES, 1], F32)
            nc.scalar.dma_start(bk_sb[:], bk_d[:])
            nc.sync.dma_start(wq_sb[:], wqT_d[:])
            nc.scalar.dma_start(bq_sb[:], bq_d[:])
            nc.sync.dma_start(wv_sb[:], wvT_d[:])
            bv_sb = cpool.tile([ES, 1], F32)
            nc.scalar.dma_start(bv_sb[:], bv_d[:])
            wo_sb = cpool.tile([ES, D], DT_OUT)
            nc.gpsimd.dma_start(wo_sb[:], woT_d[:])
            eye_sb = cpool.tile([P, P], DT_V)
            nc.scalar.dma_start(eye_sb[:], eye_d[:])

            # ---- persistent activations ---------------------------------
            qT_sb = qkv_pool.tile([P, BS], DT_QK)     # [feat 128, s 4096]
            kT_sb = qkv_pool.tile([P, BS], DT_QK)
            vT_sb = qkv_pool.tile([P, BS], DT_V)      # [feat 128, s 4096]
            # V for both heads + ones col: [tok, chunk, head, HD+1]
            v_sb = qkv_pool.tile([P, N_CH, 2, HD + 1], DT_ATT)
            oT_sb = qkv_pool.tile([P, BS], DT_OUT)    # normalized attn out, [feat, s]
            nc.vector.memset(v_sb[:, :, :, HD : HD + 1], 1.0)


            # ---- emission helpers ---------------------------------------
            strips = {}

            def emit_strip_dma(sb):
                strip = xt_pool.tile([P, N_DC, NF], DT_QK, tag="strip", name=f"strip{sb}")
                eng = nc.gpsimd if sb <= 3 else nc.sync
                eng.dma_start(strip[:], xT_d[:, sb])
                strips[sb] = strip

            def emit_qk_piece(sb, which):
                s0 = sb * NF
                strip = strips[sb]
                w_sb, bias, dst = ((wq_sb, bq_sb, qT_sb) if which == "q"
                                   else (wk_sb, bk_sb, kT_sb))
                p_ps = ps.tile([P, NF], F32, tag="misc", bufs=2, name=f"{which}{sb}_ps")
                for j in range(N_DC):
                    nc.tensor.matmul(p_ps[:], w_sb[:, j], strip[:, j],
                                     start=(j == 0), stop=(j == N_DC - 1))
                # bias-add + PSUM->SBUF evacuation on the scalar engine
                nc.scalar.add(dst[:, s0 : s0 + NF], p_ps[:], bias[:])

            def emit_v_piece(sb):
                # feat-major vT projection (N=512 matmuls, same shape as
                # q/k — the old token-major N=128 form was LDWEIGHTS-bound
                # at ~219ns per matmul, ~2x the streaming cost)
                s0 = sb * NF
                strip = strips[sb]
                v_ps = ps.tile([P, NF], F32, tag="misc", bufs=2, name=f"v{sb}_ps")
                for j in range(N_DC):
                    nc.tensor.matmul(v_ps[:], wv_sb[:, j], strip[:, j],
                                     start=(j == 0), stop=(j == N_DC - 1))
                nc.scalar.add(vT_sb[:, s0 : s0 + NF], v_ps[:], bv_sb[:])

            def emit_v_trans(sb, ss):
                # PE-transpose one 128-token chunk of vT into PV layout
                ch = sb * (NF // P) + ss
                vtp = ps.tile([P, P], DT_V, tag="misc", bufs=2, name=f"vt{ch}_ps")
                nc.tensor.transpose(vtp[:], vT_sb[:, ch * P : (ch + 1) * P],
                                    eye_sb[:])
                nc.scalar.copy(
                    v_sb[:, ch, :, 0:HD],
                    vtp[:].rearrange("p (h f) -> p h f", h=2))


            inv_sqrt_hd = _INV_SQRT_HD
            y_queue = []
            exp_acc = [0.0]

            def emit_exp(st2, pt2):
                # one engine per [128, 2, 512] tile (2 PSUM banks); round-robin
                # weighted by FRAC_DVE_EXP
                exp_acc[0] += FRAC_DVE_EXP
                if exp_acc[0] >= 1.0:
                    exp_acc[0] -= 1.0
                    nc.vector.tensor_scalar(
                        out=pt2[:].bitcast(I16), in0=st2[:],
                        scalar1=SCHRAUD_A, scalar2=SCHRAUD_B,
                        op0=mybir.AluOpType.mult, op1=mybir.AluOpType.add)
                else:
                    nc.scalar.activation(pt2[:], st2[:],
                                         mybir.ActivationFunctionType.Exp,
                                         scale=inv_sqrt_hd)

            recip_idx = [0]

            def emit_recip_chain(oraw, q0, last=False):
                # 1/rowsum on DVE (approx, ~51 ULP), then ONE partition
                # broadcast for both heads on the idle GPSIMD
                # custom-DVE ops and partition_broadcast only honor
                # partition base 0, so first move the rowsum row (partition
                # 64) to a base-0 tile with a tiny SBUF->SBUF DMA (4KB,
                # off-engine), then 1/x on DVE and broadcast on GPSIMD.
                den0 = pt_pool.tile([1, 2, NF], F32, tag="den", bufs=4)
                if last:
                    # ACT is idle at the end; its cross-partition copy is
                    # lower-latency than the DMA round trip
                    nc.scalar.copy(den0[:], oraw[HD : HD + 1, :, :])
                else:
                    nc.sync.dma_start(den0[:], oraw[HD : HD + 1, :, :])
                rcp2 = pt_pool.tile([1, 2, NF], F32, tag="rcp", bufs=4)
                nc.vector.reciprocal_approx_fast(out=rcp2[:], in_=den0[:])
                bc2 = pt_pool.tile([HD, 2, NF], F32, tag="bc", bufs=3)
                nc.gpsimd.partition_broadcast(bc2[:], rcp2[:])
                if DEBUG:
                    di = recip_idx[0]
                    recip_idx[0] += 1
                    dsl = slice(di * 2 * NF, (di + 1) * 2 * NF)
                    nc.sync.dma_start(oraw_dbg[:, dsl],
                                      oraw[:].rearrange("p h f -> p (h f)"))
                    nc.sync.dma_start(rcp_dbg[:, dsl],
                                      bc2[0:1].rearrange("p h f -> p (h f)"))
                return (oraw, bc2, q0)

            def emit_apply(oraw, bc2, q0):
                for hidx, part in ((0, 0), (1, HD)):
                    nc.vector.tensor_mul(
                        oT_sb[part : part + HD, q0 : q0 + NF],
                        oraw[0:HD, hidx], bc2[:, hidx])
                for ss in range(NF // P):
                    for ec in range(D // NF):
                        y_queue.append((q0 + ss * P, ec))

            y_acc = [0.0]

            y_dma_eng = [0]

            def emit_yproj(s0, ec, tail=False):
                y_ps = ps.tile([P, NF], F32, tag="misc", bufs=2)
                nc.tensor.matmul(y_ps[:], oT_sb[:, s0 : s0 + P],
                                 wo_sb[:, ec * NF : (ec + 1) * NF],
                                 start=True, stop=True)
                y_sb = y_pool.tile([P, NF], BF16, tag="y")
                y_acc[0] += FRAC_ACT_Y
                if y_acc[0] >= 1.0:
                    y_acc[0] -= 1.0
                    nc.scalar.copy(y_sb[:], y_ps[:])
                else:
                    nc.vector.tensor_copy(y_sb[:], y_ps[:])
                if tail:
                    # spread the final burst of y writes over both DMA-capable
                    # idle queues so the ~610ns issue cost doesn't serialize
                    eng = (nc.sync, nc.gpsimd)[y_dma_eng[0] % 2]
                    y_dma_eng[0] += 1
                else:
                    eng = nc.sync
                eng.dma_start(y_d[s0 : s0 + P, ec * NF : (ec + 1) * NF], y_sb[:])

            # ---- projections for batch 0 (k/v first; q trails as filler) -
            for sb in range(N_SB // 2):
                emit_strip_dma(sb)
                emit_qk_piece(sb, "k")
                emit_v_piece(sb)
                if sb == 0:
                    emit_qk_piece(0, "q")
                if sb > 0:
                    for ss in range(NF // P):
                        emit_v_trans(sb - 1, ss)

            # filler work queues: remaining q pieces + batch-1 projections
            # drip-feed into batch-0 attention; deferred output projections
            # drip into batch-1. q_sb{i} must complete before (b0, qc=i).
            a_queue = [("vt", 3, 0), ("vt", 3, 1), ("vt", 3, 2), ("vt", 3, 3),
                       ("q", 1), ("q", 2), ("q", 3)]
            for sb in range(N_SB // 2, N_SB):
                a_queue.append(("dma", sb))
                a_queue.append(("q", sb))
                a_queue.append(("k", sb))
                a_queue.append(("v", sb))
                for ss in range(NF // P):
                    a_queue.append(("vt", sb, ss))

            def emit_a_piece():
                piece = a_queue.pop(0)
                if piece[0] == "dma":
                    emit_strip_dma(piece[1])
                    if a_queue:
                        emit_a_piece()  # dma is async; also emit a compute piece
                elif piece[0] in ("q", "k"):
                    emit_qk_piece(piece[1], piece[0])
                elif piece[0] == "v":
                    emit_v_piece(piece[1])
                else:
                    emit_v_trans(piece[1], piece[2])

            # ---- attention: one continuous software pipeline -------------
            # Global stream over 64 ST pair-slots (8 per (b,qc) iteration);
            # PV consumption lags ST/exp by one pair and crosses iteration
            # boundaries, so the PE pipeline never drains mid-kernel.
            n_iters = B * N_QC
            total_pairs = n_iters * N_KP
            o_tiles = {}
            ptq = {}
            pending = None
            norm_state = None

            for g in range(total_pairs + 1):
                if g < total_pairs:
                    it = g // N_KP
                    kp = g % N_KP
                    b, qc = it // N_QC, it % N_QC
                    if kp == 0 and b == 1 and qc == 0:
                        while a_queue:
                            emit_a_piece()
                    q0 = b * S + qc * NF
                    st2A = ps.tile([P, 2, NF], F32, tag="st2", bufs=2)
                    st2B = ps.tile([P, 2, NF], F32, tag="st2", bufs=2)
                    for half in range(2):
                        k0 = b * S + (kp * 2 + half) * P
                        nc.tensor.matmul(st2A[:, half], kT_sb[0:HD, k0 : k0 + P],
                                         qT_sb[0:HD, q0 : q0 + NF],
                                         start=True, stop=True)
                        nc.tensor.matmul(st2B[:, half], kT_sb[HD:P, k0 : k0 + P],
                                         qT_sb[HD:P, q0 : q0 + NF],
                                         start=True, stop=True)
                    pt2A = pt_pool.tile([P, 2, NF], DT_ATT, tag="pt", bufs=10)
                    pt2B = pt_pool.tile([P, 2, NF], DT_ATT, tag="pt", bufs=10)
                    emit_exp(st2A, pt2A)
                    emit_exp(st2B, pt2B)
                    ptq[g] = (pt2A, pt2B)
                    if DEBUG and g < 8:
                        nc.sync.dma_start(
                            pt_dbg[:, g * 2 * NF : (g + 1) * 2 * NF],
                            pt2A[:].rearrange("p h f -> p (h f)"))

                    # fillers ride the ST side of the stream
                    if b == 0:
                        if a_queue:
                            emit_a_piece()
                    else:
                        npop = 3 if it >= n_iters - 2 else 2
                        for _ in range(npop):
                            # hold 8 back: they fill the PE while the final
                            # normalize chain runs
                            if len(y_queue) > 8:
                                emit_yproj(*y_queue.pop(0))
                    if kp == 1 and pending is not None:
                        norm_state = emit_recip_chain(*pending)
                        pending = None
                    if kp == 4 and norm_state is not None:
                        emit_apply(*norm_state)
                        norm_state = None

                if g >= 1:
                    pg = g - 1
                    it = pg // N_KP
                    kp = pg % N_KP
                    b, qc = it // N_QC, it % N_QC
                    q0 = b * S + qc * NF
                    if kp == 0:
                        oA_new = ps.tile([HD + 1, NF], F32, tag="o", bufs=2)
                        oB_new = ps.tile([HD + 1, NF], F32, tag="o", bufs=2)
                        o_tiles[it] = (oA_new, oB_new)
                    oA_ps, oB_ps = o_tiles[it]
                    pt2A, pt2B = ptq.pop(pg)
                    for half in range(2):
                        kb = kp * 2 + half
                        gkb = b * N_KB + kb
                        nc.tensor.matmul(oA_ps[:], v_sb[:, gkb, 0], pt2A[:, half],
                                         start=(kb == 0), stop=(kb == N_KB - 1))
                        nc.tensor.matmul(oB_ps[:], v_sb[:, gkb, 1], pt2B[:, half],
                                         start=(kb == 0), stop=(kb == N_KB - 1))
                    if kp == N_KP - 1:
                        # iteration finished: evacuate raw o + rowsum on the
                        # scalar engine, free the PSUM banks, defer norm
                        oraw = pt_pool.tile([HD + 1, 2, NF], F32, tag="oraw", bufs=3)
                        nc.scalar.copy(oraw[:, 0], oA_ps[:])
                        nc.scalar.copy(oraw[:, 1], oB_ps[:])
                        del o_tiles[it]
                        pending = (oraw, q0)

            emit_apply(*emit_recip_chain(*pending, last=True))
            for s0, ec in y_queue:
                emit_yproj(s0, ec, tail=True)

            if DEBUG:
                nc.sync.dma_start(qT_dbg[:], qT_sb[:])
                nc.sync.dma_start(kT_dbg[:], kT_sb[:])
                nc.sync.dma_start(v_dbg[:], v_sb[:].rearrange("p a h e -> p (a h e)"))
                nc.sync.dma_start(oT_dbg[:], oT_sb[:])

    nc.compile()
    return nc


def kernel(x, Wq, bq, Wk, bk, Wv, bv, Wo, bo, _trace=False):
    global last_results
    x = np.asarray(x, dtype=np.float32)
    Wq, bq = np.asarray(Wq, np.float32), np.asarray(bq, np.float32)
    Wk, bk = np.asarray(Wk, np.float32), np.asarray(bk, np.float32)
    Wv, bv = np.asarray(Wv, np.float32), np.asarray(bv, np.float32)
    Wo, bo = np.asarray(Wo, np.float32), np.asarray(bo, np.float32)

    if "nc" not in _cache:
        _cache["nc"] = _build()
    nc = _cache["nc"]

    dt_qk, dt_v, dt_out = _np_dt(DT_QK), _np_dt(DT_V), _np_dt(DT_OUT)
    _EYE = np.eye(P, dtype=dt_v)
    # [P, N_SB, N_DC, NF]: strip DMA lines are per-partition contiguous 8KB
    xT_qk = np.ascontiguousarray(
        x.reshape(N_SB, NF, N_DC, P).transpose(3, 0, 2, 1)).astype(dt_qk, copy=False)
    in_maps = []
    for c in range(NCORES):
        sl = slice(c * ES, (c + 1) * ES)
        in_maps.append({
            "xT": xT_qk,
            "wqT": np.ascontiguousarray(
                Wq[sl].T.reshape(N_DC, P, ES).transpose(1, 0, 2)).astype(dt_qk, copy=False),
            "wkT": np.ascontiguousarray(
                Wk[sl].T.reshape(N_DC, P, ES).transpose(1, 0, 2)).astype(dt_qk, copy=False),
            "wvT": np.ascontiguousarray(
                Wv[sl].T.reshape(N_DC, P, ES).transpose(1, 0, 2)).astype(dt_v, copy=False),
            "bq": np.ascontiguousarray(bq[sl, None]),
            "bk": np.ascontiguousarray(bk[sl, None]),
            "bv": np.ascontiguousarray(bv[sl, None]),
            "eye": _EYE,
            "woT": np.ascontiguousarray(Wo[:, sl].T).astype(dt_out, copy=False),
        })

    res = bass_utils.run_bass_kernel_spmd(
        nc, in_maps, core_ids=list(range(NCORES)), trace=_trace)
    last_results = res

    y = res.results[0]["y"].astype(np.float64)
    for c in range(1, NCORES):
        y += res.results[c]["y"]
    y = (y + bo).astype(np.float32)
    return y.reshape(B, S, D)


# revision 25
# speedup vs baseline: 1.0315x; 1.0059x over previous
"""Chunked (= full, non-causal) multi-head self-attention on 8 TRN2 NeuronCores.

Problem: B=2, S=2048, D=1024, H=16 heads (head_dim 64), torch-Linear-style
projections (y = x @ W.T + b), softmax attention, output projection.

Sharding: head-parallel. Core c owns heads {2c, 2c+1} = feature slice
[128c, 128c+128). Each core computes q/k/v for its slice from the full x
(replicated), runs attention for its 4 (batch, head) pairs, and produces a
partial output projection with its 128-row slice of Wo. Host sums the 8
partials (bf16) and adds bo.

Layout: scores are computed transposed, ST[k, q] (keys on partitions), so the
softmax exp output PT feeds the P@V matmul directly (contraction over k on
partitions) — x and the weights are pre-transposed AND pre-tiled on the host
so every DMA line is per-partition contiguous (the naive gather layouts ran
at 256B-1KB lines and stalled the kernel start ~15us). The two heads' K=64
score matmuls land on PE row-groups 0-1/2-3. The softmax denominator rides as
row 64 of the PV output via a ones-column appended to V (M=65). V is computed
feature-major (N=512 matmuls) then PE-transposed per 128-token chunk into the
PV layout: the direct token-major form (N=128 matmuls) was LDWEIGHTS-bound at
~219ns/matmul.

Engine balance (the v1 kernel was ACT-bound: 16.8M softmax exps at 1
elem/lane/cycle = ~147us on ScalarE alone):
  - exp is SPLIT between ACT (accurate spline exp) and DVE (Schraudolph
    bit-trick exp in bf16 space: bf16_bits(e^s) ~= round(s * 128/ln2 +
    (127*128 - 7.5)), computed as ONE fp32 tensor_scalar with int16 output,
    then the int16 tile is bitcast to bf16). Per-tile round-robin with a
    DVE-fraction knob; rel-err contribution ~1.3e-2 at 50% DVE.
  - softmax reciprocal via DVE reciprocal_approx_fast (kills the v1 ACT
    Ln/Exp chain and the activation-table monkeypatch).
  - q/k bias adds, v evacuation (bias folded into a rank-1 ones matmul) and
    o-raw evacuations moved to the otherwise-idle ScalarE (Identity/Copy are
    in the exp table set -> no table swaps); y evacuations split ACT/DVE.
  - ~20 tiny warm-up matmuls at t=0 keep the PE HAM monitor busy through the
    initial DMA wait so real matmuls run at 2.4GHz, not 1.2.

Precision: bf16 in, fp32 accumulate, bf16 partial-y out (~1.4e-2 rel err).
"""

import sys

if "/opt/trn_rl_repo" not in sys.path:
    sys.path.insert(0, "/opt/trn_rl_repo")

import numpy as np

import concourse.bacc as bacc
import concourse.mybir as mybir
import concourse.tile as tile
from concourse import bass_utils

B, S, D, H = 2, 2048, 1024, 16
HD = D // H          # 64
NCORES = 8
ES = D // NCORES     # 128 features (= 2 heads) per core
BS = B * S           # 4096 rows total

P = 128              # partitions
NF = 512             # matmul free-dim tile
N_SB = BS // NF      # 8 s-blocks of 512
N_DC = D // P        # 8 contraction chunks of 128
N_KB = S // P        # 16 key blocks of 128 per batch
N_KP = N_KB // 2     # 8 key-block PAIRS per batch
N_QC = S // NF       # 4 query chunks of 512 per batch
N_CH = BS // P       # 32 global 128-row chunks

F32 = mybir.dt.float32
BF16 = mybir.dt.bfloat16
I16 = mybir.dt.int16

DT_QK = BF16         # x/Wq/Wk inputs for q,k projections + score matmuls
DT_V = BF16          # x/Wv inputs for v projection
DT_ATT = BF16        # attention weights (exp output) and V in the P@V matmul
DT_OUT = BF16        # output projection inputs (OT, Wo)

# ---- tuning knobs -----------------------------------------------------------
FRAC_DVE_EXP = 0.56  # fraction of exp tiles done on DVE via Schraudolph
FRAC_ACT_Y = 0.42    # fraction of y evacuations done on ACT
N_WARMUP_MM = 70     # tiny matmuls at t=0 to keep the PE HAM monitor warm

# Schraudolph-in-bf16: bits = round(s * (1/sqrt(HD)) * 128/ln2 + (127*128 - C))
_INV_SQRT_HD = 1.0 / float(np.sqrt(HD))
SCHRAUD_A = float(_INV_SQRT_HD * 128.0 / np.log(2.0))
SCHRAUD_B = float(127.0 * 128.0 - 7.5)  # C=7.5 rms-optimal; HW rounds to nearest

DEBUG = False

_cache = {}
last_results = None          # test.py reads exec_time_ns off this


def _np_dt(dt):
    import ml_dtypes

    return np.dtype(ml_dtypes.bfloat16) if dt == mybir.dt.bfloat16 else np.dtype(np.float32)


def _build():
    nc = bacc.Bacc("TRN2", target_bir_lowering=False, debug=False)

    # x / W layouts pre-arranged on the host so every DMA line is
    # per-partition contiguous (8KB strips, 2KB weights): the naive
    # [D, BS] gather ran at ~256B-1KB per line and stalled the PE ~15us
    # at kernel start.
    xT_d = nc.dram_tensor("xT", [P, N_SB, N_DC, NF], DT_QK, kind="ExternalInput")
    wqT_d = nc.dram_tensor("wqT", [P, N_DC, ES], DT_QK, kind="ExternalInput")
    wkT_d = nc.dram_tensor("wkT", [P, N_DC, ES], DT_QK, kind="ExternalInput")
    wvT_d = nc.dram_tensor("wvT", [P, N_DC, ES], DT_V, kind="ExternalInput")
    bq_d = nc.dram_tensor("bq", [ES, 1], F32, kind="ExternalInput")
    bk_d = nc.dram_tensor("bk", [ES, 1], F32, kind="ExternalInput")
    bv_d = nc.dram_tensor("bv", [ES, 1], F32, kind="ExternalInput")
    eye_d = nc.dram_tensor("eye", [P, P], DT_V, kind="ExternalInput")
    woT_d = nc.dram_tensor("woT", [ES, D], DT_OUT, kind="ExternalInput")
    y_d = nc.dram_tensor("y", [BS, D], BF16, kind="ExternalOutput")
    if DEBUG:
        qT_dbg = nc.dram_tensor("qT_dbg", [P, BS], DT_QK, kind="ExternalOutput")
        kT_dbg = nc.dram_tensor("kT_dbg", [P, BS], DT_QK, kind="ExternalOutput")
        v_dbg = nc.dram_tensor("v_dbg", [P, N_CH * 2 * (HD + 1)], DT_ATT, kind="ExternalOutput")
        oT_dbg = nc.dram_tensor("oT_dbg", [P, BS], DT_OUT, kind="ExternalOutput")
        oraw_dbg = nc.dram_tensor("oraw_dbg", [HD + 1, 16 * 2 * NF], F32, kind="ExternalOutput")
        rcp_dbg = nc.dram_tensor("rcp_dbg", [1, 16 * 2 * NF], F32, kind="ExternalOutput")
        pt_dbg = nc.dram_tensor("pt_dbg", [P, 8 * 2 * NF], DT_ATT, kind="ExternalOutput")

    with tile.TileContext(nc) as tc:
        with tc.tile_pool(name="const", bufs=1) as cpool, \
             tc.tile_pool(name="xt", bufs=3) as xt_pool, \
             tc.tile_pool(name="qkv", bufs=1) as qkv_pool, \
             tc.tile_pool(name="pt", bufs=10) as pt_pool, \
             tc.tile_pool(name="ysb", bufs=6) as y_pool, \
             tc.tile_pool(name="ps", bufs=1, space="PSUM") as ps:

            # ---- PE warm-up: tiny matmuls while the first DMAs land -------
            dummy_w = cpool.tile([1, P], DT_QK)
            nc.vector.memset(dummy_w[:], 0.0)
            for _ in range(N_WARMUP_MM):
                warm_ps = ps.tile([P, P], F32, tag="misc", bufs=2)
                nc.tensor.matmul(warm_ps[:], dummy_w[:], dummy_w[:],
                                 start=True, stop=True)

            # ---- constants / weights ------------------------------------
            # (first-strip DMA is issued before these from the gpsimd queue
            # inside the batch-0 loop; weights ride the sync/scalar queues)
            wk_sb = cpool.tile([P, N_DC, ES], DT_QK)
            wq_sb = cpool.tile([P, N_DC, ES], DT_QK)
            wv_sb = cpool.tile([P, N_DC, ES], DT_V)
            nc.sync.dma_start(wk_sb[:], wkT_d[:])
            bk_sb = cpool.tile([ES, 1], F32)
            bq_sb = cpool.tile([ES, 1], F32)
            nc.scalar.dma_start(bk_sb[:], bk_d[:])
            nc.sync.dma_start(wq_sb[:], wqT_d[:])
            nc.scalar.dma_start(bq_sb[:], bq_d[:])
            nc.sync.dma_start(wv_sb[:], wvT_d[:])
            bv_sb = cpool.tile([ES, 1], F32)
            nc.scalar.dma_start(bv_sb[:], bv_d[:])
            wo_sb = cpool.tile([ES, D], DT_OUT)
            nc.gpsimd.dma_start(wo_sb[:], woT_d[:])
            eye_sb = cpool.tile([P, P], DT_V)
            nc.scalar.dma_start(eye_sb[:], eye_d[:])

            # ---- persistent activations ---------------------------------
            qT_sb = qkv_pool.tile([P, BS], DT_QK)     # [feat 128, s 4096]
            kT_sb = qkv_pool.tile([P, BS], DT_QK)
            vT_sb = qkv_pool.tile([P, BS], DT_V)      # [feat 128, s 4096]
            # V for both heads + ones col: [tok, chunk, head, HD+1]
            v_sb = qkv_pool.tile([P, N_CH, 2, HD + 1], DT_ATT)
            oT_sb = qkv_pool.tile([P, BS], DT_OUT)    # normalized attn out, [feat, s]
            nc.vector.memset(v_sb[:, :, :, HD : HD + 1], 1.0)


            # ---- emission helpers ---------------------------------------
            strips = {}

            def emit_strip_dma(sb):
                strip = xt_pool.tile([P, N_DC, NF], DT_QK, tag="strip", name=f"strip{sb}")
                eng = nc.gpsimd if sb <= 3 else nc.sync
                eng.dma_start(strip[:], xT_d[:, sb])
                strips[sb] = strip

            def emit_qk_piece(sb, which):
                s0 = sb * NF
                strip = strips[sb]
                w_sb, bias, dst = ((wq_sb, bq_sb, qT_sb) if which == "q"
                                   else (wk_sb, bk_sb, kT_sb))
                p_ps = ps.tile([P, NF], F32, tag="misc", bufs=2, name=f"{which}{sb}_ps")
                for j in range(N_DC):
                    nc.tensor.matmul(p_ps[:], w_sb[:, j], strip[:, j],
                                     start=(j == 0), stop=(j == N_DC - 1))
                # bias-add + PSUM->SBUF evacuation on the scalar engine
                nc.scalar.add(dst[:, s0 : s0 + NF], p_ps[:], bias[:])

            def emit_v_piece(sb):
                # feat-major vT projection (N=512 matmuls, same shape as
                # q/k — the old token-major N=128 form was LDWEIGHTS-bound
                # at ~219ns per matmul, ~2x the streaming cost)
                s0 = sb * NF
                strip = strips[sb]
                v_ps = ps.tile([P, NF], F32, tag="misc", bufs=2, name=f"v{sb}_ps")
                for j in range(N_DC):
                    nc.tensor.matmul(v_ps[:], wv_sb[:, j], strip[:, j],
                                     start=(j == 0), stop=(j == N_DC - 1))
                nc.scalar.add(vT_sb[:, s0 : s0 + NF], v_ps[:], bv_sb[:])

            def emit_v_trans(sb, ss):
                # PE-transpose one 128-token chunk of vT into PV layout
                ch = sb * (NF // P) + ss
                vtp = ps.tile([P, P], DT_V, tag="misc", bufs=2, name=f"vt{ch}_ps")
                nc.tensor.transpose(vtp[:], vT_sb[:, ch * P : (ch + 1) * P],
                                    eye_sb[:])
                nc.scalar.copy(
                    v_sb[:, ch, :, 0:HD],
                    vtp[:].rearrange("p (h f) -> p h f", h=2))


            inv_sqrt_hd = _INV_SQRT_HD
            y_queue = []
            exp_acc = [0.0]

            def emit_exp(st2, pt2):
                # one engine per [128, 2, 512] tile (2 PSUM banks); round-robin
                # weighted by FRAC_DVE_EXP
                exp_acc[0] += FRAC_DVE_EXP
                if exp_acc[0] >= 1.0:
                    exp_acc[0] -= 1.0
                    nc.vector.tensor_scalar(
                        out=pt2[:].bitcast(I16), in0=st2[:],
                        scalar1=SCHRAUD_A, scalar2=SCHRAUD_B,
                        op0=mybir.AluOpType.mult, op1=mybir.AluOpType.add)
                else:
                    nc.scalar.activation(pt2[:], st2[:],
                                         mybir.ActivationFunctionType.Exp,
                                         scale=inv_sqrt_hd)

            recip_idx = [0]

            def emit_recip_chain(oraw, q0, last=False):
                # 1/rowsum on DVE (approx, ~51 ULP), then ONE partition
                # broadcast for both heads on the idle GPSIMD
                # custom-DVE ops and partition_broadcast only honor
                # partition base 0, so first move the rowsum row (partition
                # 64) to a base-0 tile with a tiny SBUF->SBUF DMA (4KB,
                # off-engine), then 1/x on DVE and broadcast on GPSIMD.
                den0 = pt_pool.tile([1, 2, NF], F32, tag="den", bufs=4)
                if last:
                    # ACT is idle at the end; its cross-partition copy is
                    # lower-latency than the DMA round trip
                    nc.scalar.copy(den0[:], oraw[HD : HD + 1, :, :])
                else:
                    nc.sync.dma_start(den0[:], oraw[HD : HD + 1, :, :])
                rcp2 = pt_pool.tile([1, 2, NF], F32, tag="rcp", bufs=4)
                nc.vector.reciprocal_approx_fast(out=rcp2[:], in_=den0[:])
                bc2 = pt_pool.tile([HD, 2, NF], F32, tag="bc", bufs=3)
                nc.gpsimd.partition_broadcast(bc2[:], rcp2[:])
                if DEBUG:
                    di = recip_idx[0]
                    recip_idx[0] += 1
                    dsl = slice(di * 2 * NF, (di + 1) * 2 * NF)
                    nc.sync.dma_start(oraw_dbg[:, dsl],
                                      oraw[:].rearrange("p h f -> p (h f)"))
                    nc.sync.dma_start(rcp_dbg[:, dsl],
                                      bc2[0:1].rearrange("p h f -> p (h f)"))
                return (oraw, bc2, q0)

            def emit_apply(oraw, bc2, q0):
                for hidx, part in ((0, 0), (1, HD)):
                    nc.vector.tensor_mul(
                        oT_sb[part : part + HD, q0 : q0 + NF],
                        oraw[0:HD, hidx], bc2[:, hidx])
                for ss in range(NF // P):
                    for ec in range(D // NF):
                        y_queue.append((q0 + ss * P, ec))

            y_acc = [0.0]

            y_dma_eng = [0]

            def emit_yproj(s0, ec, tail=False):
                y_ps = ps.tile([P, NF], F32, tag="misc", bufs=2)
                nc.tensor.matmul(y_ps[:], oT_sb[:, s0 : s0 + P],
                                 wo_sb[:, ec * NF : (ec + 1) * NF],
                                 start=True, stop=True)
                y_sb = y_pool.tile([P, NF], BF16, tag="y")
                y_acc[0] += FRAC_ACT_Y
                if y_acc[0] >= 1.0:
                    y_acc[0] -= 1.0
                    nc.scalar.copy(y_sb[:], y_ps[:])
                else:
                    nc.vector.tensor_copy(y_sb[:], y_ps[:])
                if tail:
                    # spread the final burst of y writes over both DMA-capable
                    # idle queues so the ~610ns issue cost doesn't serialize
                    eng = (nc.sync, nc.gpsimd)[y_dma_eng[0] % 2]
                    y_dma_eng[0] += 1
                else:
                    eng = nc.sync
                eng.dma_start(y_d[s0 : s0 + P, ec * NF : (ec + 1) * NF], y_sb[:])

            # ---- projections for batch 0 (k/v first; q trails as filler) -
            for sb in range(N_SB // 2):
                emit_strip_dma(sb)
                emit_qk_piece(sb, "k")
                emit_v_piece(sb)
                if sb == 0:
                    emit_qk_piece(0, "q")
                if sb > 0:
                    for ss in range(NF // P):
                        emit_v_trans(sb - 1, ss)

            # filler work queues: remaining q pieces + batch-1 projections
            # drip-feed into batch-0 attention; deferred output projections
            # drip into batch-1. q_sb{i} must complete before (b0, qc=i).
            a_queue = [("vt", 3, 0), ("vt", 3, 1), ("vt", 3, 2), ("vt", 3, 3),
                       ("q", 1), ("q", 2), ("q", 3)]
            for sb in range(N_SB // 2, N_SB):
                a_queue.append(("dma", sb))
                a_queue.append(("q", sb))
                a_queue.append(("k", sb))
                a_queue.append(("v", sb))
                for ss in range(NF // P):
                    a_queue.append(("vt", sb, ss))

            def emit_a_piece():
                piece = a_queue.pop(0)
                if piece[0] == "dma":
                    emit_strip_dma(piece[1])
                    if a_queue:
                        emit_a_piece()  # dma is async; also emit a compute piece
                elif piece[0] in ("q", "k"):
                    emit_qk_piece(piece[1], piece[0])
                elif piece[0] == "v":
                    emit_v_piece(piece[1])
                else:
                    emit_v_trans(piece[1], piece[2])

            # ---- attention: one continuous software pipeline -------------
            # Global stream over 64 ST pair-slots (8 per (b,qc) iteration);
            # PV consumption lags ST/exp by one pair and crosses iteration
            # boundaries, so the PE pipeline never drains mid-kernel.
            n_iters = B * N_QC
            total_pairs = n_iters * N_KP
            o_tiles = {}
            ptq = {}
            pending = None
            norm_state = None

            for g in range(total_pairs + 1):
                if g < total_pairs:
                    it = g // N_KP
                    kp = g % N_KP
                    b, qc = it // N_QC, it % N_QC
                    if kp == 0 and b == 1 and qc == 0:
                        while a_queue:
                            emit_a_piece()
                    q0 = b * S + qc * NF
                    st2A = ps.tile([P, 2, NF], F32, tag="st2", bufs=2)
                    st2B = ps.tile([P, 2, NF], F32, tag="st2", bufs=2)
                    for half in range(2):
                        k0 = b * S + (kp * 2 + half) * P
                        nc.tensor.matmul(st2A[:, half], kT_sb[0:HD, k0 : k0 + P],
                                         qT_sb[0:HD, q0 : q0 + NF],
                                         start=True, stop=True)
                        nc.tensor.matmul(st2B[:, half], kT_sb[HD:P, k0 : k0 + P],
                                         qT_sb[HD:P, q0 : q0 + NF],
                                         start=True, stop=True)
                    pt2A = pt_pool.tile([P, 2, NF], DT_ATT, tag="pt", bufs=10)
                    pt2B = pt_pool.tile([P, 2, NF], DT_ATT, tag="pt", bufs=10)
                    emit_exp(st2A, pt2A)
                    emit_exp(st2B, pt2B)
                    ptq[g] = (pt2A, pt2B)
                    if DEBUG and g < 8:
                        nc.sync.dma_start(
                            pt_dbg[:, g * 2 * NF : (g + 1) * 2 * NF],
                            pt2A[:].rearrange("p h f -> p (h f)"))

                    # fillers ride the ST side of the stream
                    if b == 0:
                        if a_queue:
                            emit_a_piece()
                    else:
                        npop = 3 if it >= n_iters - 2 else 2
                        for _ in range(npop):
                            # hold 8 back: they fill the PE while the final
                            # normalize chain runs
                            if len(y_queue) > 8:
                                emit_yproj(*y_queue.pop(0))
                    if kp == 1 and pending is not None:
                        norm_state = emit_recip_chain(*pending)
                        pending = None
                    if kp == 4 and norm_state is not None:
                        emit_apply(*norm_state)
                        norm_state = None

                if g >= 1:
                    pg = g - 1
                    it = pg // N_KP
                    kp = pg % N_KP
                    b, qc = it // N_QC, it % N_QC
                    q0 = b * S + qc * NF
                    if kp == 0:
                        oA_new = ps.tile([HD + 1, NF], F32, tag="o", bufs=2)
                        oB_new = ps.tile([HD + 1, NF], F32, tag="o", bufs=2)
                        o_tiles[it] = (oA_new, oB_new)
                    oA_ps, oB_ps = o_tiles[it]
                    pt2A, pt2B = ptq.pop(pg)
                    for half in range(2):
                        kb = kp * 2 + half
                        gkb = b * N_KB + kb
                        nc.tensor.matmul(oA_ps[:], v_sb[:, gkb, 0], pt2A[:, half],
                                         start=(kb == 0), stop=(kb == N_KB - 1))
                        nc.tensor.matmul(oB_ps[:], v_sb[:, gkb, 1], pt2B[:, half],
                                         start=(kb == 0), stop=(kb == N_KB - 1))
                    if kp == N_KP - 1:
                        # iteration finished: evacuate raw o + rowsum on the
                        # scalar engine, free the PSUM banks, defer norm
                        oraw = pt_pool.tile([HD + 1, 2, NF], F32, tag="oraw", bufs=3)
                        nc.scalar.copy(oraw[:, 0], oA_ps[:])
                        nc.scalar.copy(oraw[:, 1], oB_ps[:])
                        del o_tiles[it]
                        pending = (oraw, q0)

            emit_apply(*emit_recip_chain(*pending, last=True))
            for s0, ec in y_queue:
                emit_yproj(s0, ec, tail=True)

            if DEBUG:
                nc.sync.dma_start(qT_dbg[:], qT_sb[:])
                nc.sync.dma_start(kT_dbg[:], kT_sb[:])
                nc.sync.dma_start(v_dbg[:], v_sb[:].rearrange("p a h e -> p (a h e)"))
                nc.sync.dma_start(oT_dbg[:], oT_sb[:])

    nc.compile()
    return nc


def kernel(x, Wq, bq, Wk, bk, Wv, bv, Wo, bo, _trace=False):
    global last_results
    x = np.asarray(x, dtype=np.float32)
    Wq, bq = np.asarray(Wq, np.float32), np.asarray(bq, np.float32)
    Wk, bk = np.asarray(Wk, np.float32), np.asarray(bk, np.float32)
    Wv, bv = np.asarray(Wv, np.float32), np.asarray(bv, np.float32)
    Wo, bo = np.asarray(Wo, np.float32), np.asarray(bo, np.float32)

    if "nc" not in _cache:
        _cache["nc"] = _build()
    nc = _cache["nc"]

    dt_qk, dt_v, dt_out = _np_dt(DT_QK), _np_dt(DT_V), _np_dt(DT_OUT)
    _EYE = np.eye(P, dtype=dt_v)
    # [P, N_SB, N_DC, NF]: strip DMA lines are per-partition contiguous 8KB
    xT_qk = np.ascontiguousarray(
        x.reshape(N_SB, NF, N_DC, P).transpose(3, 0, 2, 1)).astype(dt_qk, copy=False)
    in_maps = []
    for c in range(NCORES):
        sl = slice(c * ES, (c + 1) * ES)
        in_maps.append({
            "xT": xT_qk,
            "wqT": np.ascontiguousarray(
                Wq[sl].T.reshape(N_DC, P, ES).transpose(1, 0, 2)).astype(dt_qk, copy=False),
            "wkT": np.ascontiguousarray(
                Wk[sl].T.reshape(N_DC, P, ES).transpose(1, 0, 2)).astype(dt_qk, copy=False),
            "wvT": np.ascontiguousarray(
                Wv[sl].T.reshape(N_DC, P, ES).transpose(1, 0, 2)).astype(dt_v, copy=False),
            "bq": np.ascontiguousarray(bq[sl, None]),
            "bk": np.ascontiguousarray(bk[sl, None]),
            "bv": np.ascontiguousarray(bv[sl, None]),
            "eye": _EYE,
            "woT": np.ascontiguousarray(Wo[:, sl].T).astype(dt_out, copy=False),
        })

    res = bass_utils.run_bass_kernel_spmd(
        nc, in_maps, core_ids=list(range(NCORES)), trace=_trace)
    last_results = res

    y = res.results[0]["y"].astype(np.float64)
    for c in range(1, NCORES):
        y += res.results[c]["y"]
    y = (y + bo).astype(np.float32)
    return y.reshape(B, S, D)
